# revision 50
# baseline (speedup 1.0000x reference)
"""Trainium2 Bass kernel for nn_MACAM (cross-attn modulation + instance norm).

Strategy: pure data parallel - batch B=16 sharded 2 samples per core over 8
NeuronCores.  Per sample the conv + fc_k are folded on the host into a single
matrix Mf = fc_k_w.T @ conv_w so the device computes
    kcT = Mf.T @ ws.T + c0,   attn = kcT.T @ h + kb        (kb host-folded)
The argmax/gather runs on-chip as a one-hot matmul.  The beta/gamma map
matmuls run with K=64 (attn lives on partitions 0-63), bf16 weights and
moving operand.  InstanceNorm scale `s` folds at piece level via
scalar_tensor_tensor  tmp = (gamma_map * s) * h  on DVE, and the final add
runs as a PE identity-accumulate into the beta-map PSUM (beta2 =
beta + t2*gamma folded on-chip).  Output is written bf16 and upcast on the
host.  attn pieces are interleaved with modulation pieces so the PE never
idles long enough for HAM to re-throttle the clock.
"""

import os
import sys

os.environ.setdefault("MYCRO_LOCAL_CACHE", "1")
sys.path.insert(0, "/opt/trn_rl_repo")

import numpy as np

import concourse.bacc as bacc
import concourse.bass as bass
import concourse.mybir as mybir
import concourse.tile as tile
from concourse.bass_utils import run_bass_kernel_spmd

N_CORES = 8
B, C, H, W = 16, 512, 64, 64
HW = H * W
L, D, Q = 64, 512, 512
S = B // N_CORES          # samples per core
EPS = 1e-5
NP = 8                    # HW pieces of 512
NC4 = 4                   # channel chunks of 128

f32 = mybir.dt.float32
f32r = mybir.dt.float32r
bf16 = mybir.dt.bfloat16
AF = mybir.ActivationFunctionType
ALU = mybir.AluOpType
AX = mybir.AxisListType


def _build_program():
    nc = bacc.Bacc("TRN2", target_bir_lowering=False, debug=False,
                   num_devices=N_CORES)
    dt_ = nc.dram_tensor
    h_d = dt_("h", [S, C, H, W], f32, kind="ExternalInput").ap()
    u8 = mybir.dt.uint8
    WB = 16128
    SB = 3584
    wblk_d = dt_("wblk", [128, WB], u8, kind="ExternalInput").ap()
    spk_d = dt_("spk", [S, 128, SB], u8, kind="ExternalInput").ap()
    fb_d = dt_("fc_b_row", [1, 2 * C], f32, kind="ExternalInput").ap()
    ones1_d = dt_("ones1", [1, L], f32, kind="ExternalInput").ap()
    sel8_d = dt_("sel8", [8, 8 * L], f32, kind="ExternalInput").ap()
    out_d = dt_("out", [S, C, HW], bf16, kind="ExternalOutput").ap()

    h_v = h_d.rearrange("s (n p) a b -> s n p (a b)", p=128)     # [S,4,128,4096]
    out_v = out_d.rearrange("s (n p) q -> s n p q", p=128)

    with tile.TileContext(nc) as tc:
        with (
            tc.tile_pool(name="wpool", bufs=1) as wpool,
            tc.tile_pool(name="hpool", bufs=8) as hpool,
            tc.tile_pool(name="attnpool", bufs=2) as attnpool,
            tc.tile_pool(name="spool", bufs=2) as spool,
            tc.tile_pool(name="piece", bufs=7) as piece,
            tc.tile_pool(name="ps_bg", bufs=1, space="PSUM") as ps_bg,
            tc.tile_pool(name="ps_ring", bufs=7, space="PSUM") as ps_ring,
        ):
            # ---- packed input DMAs: one big-packet block for the
            # persistent weights, one per-sample block for the smalls, so
            # they get a fair DMA round-robin share against the h stream --
            wblk = wpool.tile([128, WB], mybir.dt.uint8, tag="wblk")
            nc.sync.dma_start(wblk[:], wblk_d)
            mf_t = [wblk[:, j * 1024:(j + 1) * 1024].bitcast(bf16)
                    for j in range(4)]
            fw_t = [wblk[:, 4096 + j * 2048:4096 + (j + 1) * 2048].bitcast(bf16)
                    for j in range(4)]

            ones64 = wblk[0:L, 14336:14592].bitcast(f32)
            nid64 = wblk[0:L, 14592:14848].bitcast(f32)
            id64b = wblk[0:L, 14848:14976].bitcast(bf16)
            id128b = wblk[:, 15232:15488].bitcast(bf16)
            id128 = wblk[:, 15488:16000].bitcast(f32)
            tinyb = wblk[:, 16000:16064].bitcast(f32)
            c0_col = tinyb[:, 0:4]
            inw_col = tinyb[:, 4:8]
            inb_col = tinyb[:, 8:12]
            epz = tinyb[:, 12:14]
            kbs = [tinyb[0:L, 14:15], tinyb[0:L, 15:16]]
            wsT4s, wsT4bs, wtT4s, wts, hts = [], [], [], [], []
            for s in range(S):
                spk = spool.tile([128, SB], mybir.dt.uint8, tag="spk")
                nc.sync.dma_start(spk[:], spk_d[s])
                wsT4s.append(spk[:, 0:1024].bitcast(f32))
                wtT4s.append(spk[:, 1024:2048].bitcast(f32))
                wsT4bs.append(spk[:, 2048:2560].bitcast(bf16))
                wts.append(spk[0:L, 2560:3584].bitcast(bf16))
            fcb_row = wpool.tile([1, 2 * C], f32r, tag="fcb")
            nc.sync.dma_start(fcb_row[:], fb_d.bitcast(f32r))
            ones1r = wpool.tile([1, L], f32r, tag="ones1r")
            nc.sync.dma_start(ones1r[:], ones1_d.bitcast(f32r))
            sel8 = wpool.tile([8, 8 * L], f32r, tag="sel8")
            nc.sync.dma_start(sel8[:], sel8_d.bitcast(f32r))
            for s in range(S):
                h_t = []
                for cc in range(NC4):
                    t = hpool.tile([128, HW], bf16, tag="h", name=f"h{s}{cc}")
                    nc.gpsimd.dma_start(t[:], h_v[s, cc])
                    h_t.append(t)
                hts.append(h_t)

            st = [dict() for _ in range(S)]

            def prologue_scores(s):
                wsT4, wtT4, wt_sb = wsT4s[s], wtT4s[s], wts[s]
                # one PSUM bank holds the whole small path as scratch:
                #   [0:64, 0:64]    scores       [0:1, 64:128] colsum
                #   [0:64,128:192]  left         [0:64,192:224] PT (bf16)
                #   [0:8, 224:352]  stT          [0:128,384:448] kcT (4x)
                #   [0:128,448:512] waT (4x)
                # then gamma -> [0:64, 0:512], copied out, then beta.
                scr = ps_bg.tile([128, 512], f32, tag="bg")
                st[s]["scr"] = scr

                scores_ps = scr[0:L, 0:L]
                for j in range(4):
                    nc.tensor.matmul(
                        scores_ps,
                        wsT4[:, j * L:(j + 1) * L], wtT4[:, j * L:(j + 1) * L],
                        start=(j == 0), stop=(j == 3))
                scores_sb = spool.tile([L, L], f32, tag="scores_sb")
                nc.scalar.copy(scores_sb[:], scores_ps)
                colsum_ps = scr[0:1, 64:64 + L]
                nc.tensor.matmul(colsum_ps, ones64[:, 0:1], scores_sb[:],
                                 start=True, stop=True)
                colsum_row = spool.tile([1, L], f32, tag="colsum")
                nc.scalar.copy(colsum_row[:], colsum_ps)
                left_ps = scr[0:L, 128:128 + L]
                nc.tensor.matmul(left_ps, ones64[0:1, :], colsum_row[:],
                                 start=True, stop=False)
                nc.tensor.matmul(left_ps, nid64[:], scores_sb[:],
                                 start=False, stop=True)
                rowmax = spool.tile([L, 1], f32, tag="rowmax")
                nc.vector.tensor_reduce(rowmax[:], left_ps, AX.X, ALU.max)
                P_sb = spool.tile([L, L], bf16, tag="P_sb")
                nc.vector.tensor_scalar(P_sb[:], left_ps, rowmax[:], None,
                                        ALU.is_equal)
                PT_ps = scr[0:L, 192:224].bitcast(bf16)
                nc.tensor.transpose(PT_ps, P_sb[:], id64b[:])
                PT_sb = spool.tile([L, L], bf16, tag="PT_sb")
                nc.scalar.copy(PT_sb[:], PT_ps)

                # kcT = Mf.T @ ws.T + c0
                kcT_sb = spool.tile([128, L * NC4], bf16, tag="kcT_sb")
                st[s]["kcT"] = kcT_sb
                wsT4b = wsT4bs[s]
                for cc in range(NC4):
                    kcT_ps = scr[0:128, 384:384 + L]
                    for j in range(4):
                        nc.tensor.matmul(
                            kcT_ps, mf_t[j][:, cc * 128:(cc + 1) * 128],
                            wsT4b[:, j * L:(j + 1) * L],
                            start=(j == 0), stop=(j == 3))
                    nc.scalar.activation(
                        kcT_sb[:, cc * L:(cc + 1) * L],
                        kcT_ps, AF.Identity, bias=c0_col[:, cc:cc + 1])

                # w_allocT
                waT_sb = spool.tile([128, 4 * L], bf16, tag="waT_sb")
                st[s]["waT"] = waT_sb
                for j in range(4):
                    waT_ps = scr[0:128, 448:448 + L]
                    nc.tensor.matmul(waT_ps, wt_sb[:, j * 128:(j + 1) * 128],
                                     PT_sb[:], start=True, stop=True)
                    nc.scalar.copy(waT_sb[:, j * L:(j + 1) * L], waT_ps)

            def stats_slice(s, cc, k):
                if "st_col" not in st[s]:
                    st[s]["st_col"] = spool.tile([128, 8], f32, tag="st_col", name="st_col")
                    st[s]["st6"] = {}
                if cc not in st[s]["st6"]:
                    st[s]["st6"][cc] = spool.tile([128, 48], f32, tag="st6", name="st6")
                st6 = st[s]["st6"][cc]
                nc.vector.bn_stats(
                    st6[:, k * 6:(k + 1) * 6],
                    hts[s][cc][:, k * 512:(k + 1) * 512])

            def stats_aggr(s, cc):
                st_col, st6 = st[s]["st_col"], st[s]["st6"][cc]
                mv = spool.tile([128, 2], f32, tag="mv")
                nc.vector.bn_aggr(mv[:], st6[:])
                sd = spool.tile([128, 1], f32, tag="sd")
                nc.scalar.activation(sd[:], mv[:, 1:2], AF.Sqrt, bias=epz[:, 0:1])
                rs = spool.tile([128, 1], f32, tag="rs")
                nc.vector.reciprocal(rs[:], sd[:])
                nc.vector.tensor_tensor(
                    st_col[:, cc:cc + 1], rs[:], inw_col[:, cc:cc + 1],
                    ALU.mult)
                ms = spool.tile([128, 1], f32, tag="ms")
                nc.vector.tensor_tensor(ms[:], mv[:, 0:1],
                                        st_col[:, cc:cc + 1], ALU.mult)
                nc.vector.tensor_tensor(st_col[:, 4 + cc:5 + cc],
                                        inb_col[:, cc:cc + 1], ms[:],
                                        ALU.subtract)

            def prologue_fold(s):
                scr, st_col, waT_sb = st[s]["scr"], st[s]["st_col"], st[s]["waT"]
                # t2 broadcast to [64,512]
                stT_ps = scr[0:8, 224:352]
                nc.tensor.transpose(stT_ps, st_col[:], id128[:])
                st8r = spool.tile([8, 128], f32r, tag="st8r")
                nc.scalar.copy(st8r[:], stT_ps)
                t2m_t = ps_ring.tile([128, 512], f32, tag="ring")
                t2m_ps = t2m_t[0:L, :]
                for j in range(4):
                    nc.tensor.matmul(t2m_ps[:, j * 128:(j + 1) * 128],
                                     sel8[:, (4 + j) * L:(5 + j) * L], st8r[:],
                                     start=True, stop=True)
                t2m_sb = spool.tile([L, C], bf16, tag="t2m_sb")
                nc.scalar.copy(t2m_sb[:], t2m_ps)
                smap_t = ps_ring.tile([128, 512], f32, tag="ring")
                smap_ps = smap_t[0:L, :]
                for j in range(4):
                    nc.tensor.matmul(smap_ps[:, j * 128:(j + 1) * 128],
                                     sel8[:, j * L:(j + 1) * L], st8r[:],
                                     start=True, stop=True)
                smap_sb = spool.tile([L, C], bf16, tag="smap_sb")
                nc.scalar.copy(smap_sb[:], smap_ps)

                # gamma then beta, sequentially through scr[0:64, :]
                for j in range(4):
                    nc.tensor.matmul(
                        scr[0:L, :], waT_sb[:, j * L:(j + 1) * L],
                        fw_t[j][:, C:2 * C], start=(j == 0), stop=False)
                nc.tensor.matmul(scr[0:L, :], ones1r[:], fcb_row[:, C:2 * C],
                                 start=False, stop=True)
                # gamma2 = s * gamma ;  tg = t2 * gamma  (both from PSUM)
                gbg = spool.tile([L, C], bf16, tag="gbg")
                nc.vector.tensor_tensor(gbg[:], scr[0:L, :], smap_sb[:],
                                        ALU.mult)
                st[s]["gbg"] = gbg
                nc.vector.tensor_tensor(t2m_sb[:], scr[0:L, :], t2m_sb[:],
                                        ALU.mult)
                for j in range(4):
                    nc.tensor.matmul(
                        scr[0:L, :], waT_sb[:, j * L:(j + 1) * L],
                        fw_t[j][:, 0:C], start=(j == 0), stop=False)
                nc.tensor.matmul(scr[0:L, :], ones1r[:], fcb_row[:, 0:C],
                                 start=False, stop=True)
                # beta2 = beta + t2*gamma
                gbb = spool.tile([L, C], bf16, tag="gbb")   # beta2
                nc.vector.tensor_tensor(gbb[:], scr[0:L, :], t2m_sb[:], ALU.add)
                st[s]["gbb"] = gbb
                if "attn_sb" not in st[s]:
                    st[s]["attn_sb"] = attnpool.tile(
                        [L, HW], bf16, tag="attn_sb", name="attn_sb")
                st[s]["pend"] = []

            def do_attn(s, pp):
                kcT_sb, h_t, attn_sb = st[s]["kcT"], hts[s], st[s]["attn_sb"]
                attn_t = ps_ring.tile([128, 512], f32, tag="ring")
                attn_ps = attn_t[0:L, :]
                for cc in range(NC4):
                    nc.tensor.matmul(
                        attn_ps, kcT_sb[:, cc * L:(cc + 1) * L],
                        h_t[cc][:, pp * 512:(pp + 1) * 512],
                        start=(cc == 0), stop=(cc == 3))
                nc.scalar.activation(attn_sb[:, pp * 512:(pp + 1) * 512],
                                     attn_ps, AF.Identity, bias=kbs[s][:])

            def finish_piece(s, cc, pp, bm_ps, tmp):
                nc.tensor.matmul(bm_ps[:], id128b[:], tmp[:],
                                 start=False, stop=True)
                outp = piece.tile([128, 512], bf16, tag="outp")
                nc.scalar.copy(outp[:], bm_ps[:])
                nc.sync.dma_start(
                    out_v[s, cc][:, pp * 512:(pp + 1) * 512], outp[:])

            def do_maps(s, pp, side=None):
                attn_sb, gbg, gbb = st[s]["attn_sb"], st[s]["gbg"], st[s]["gbb"]
                h_t, pend = hts[s], st[s]["pend"]
                aps = attn_sb[:, pp * 512:(pp + 1) * 512]
                # phase 1: all gamma-map matmuls + vector multiplies, so the
                # multiplies overlap the beta-map matmuls of phase 2
                gms, tmps = [], []
                for cc in range(NC4):
                    gm_ps = ps_ring.tile([128, 512], f32, tag="ring")
                    nc.tensor.matmul(
                        gm_ps[:], gbg[:, cc * 128:(cc + 1) * 128],
                        aps, start=True, stop=True)
                    tmp = piece.tile([128, 512], bf16, tag="tmp")
                    if cc == 3:
                        gmc = piece.tile([128, 512], bf16, tag="gmc")
                        nc.scalar.copy(gmc[:], gm_ps[:])
                        nc.gpsimd.tensor_tensor(
                            tmp[:], gmc[:],
                            h_t[cc][:, pp * 512:(pp + 1) * 512], ALU.mult)
                    else:
                        nc.vector.tensor_tensor(
                            tmp[:], gm_ps[:],
                            h_t[cc][:, pp * 512:(pp + 1) * 512], ALU.mult)
                    tmps.append(tmp)
                    if side:
                        side.pop(0)()
                        if side and cc % 2 == 1:
                            side.pop(0)()
                # phase 2: beta-map matmuls + accumulate + copy out
                for cc in range(NC4):
                    bm_ps = ps_ring.tile([128, 512], f32, tag="ring")
                    nc.tensor.matmul(
                        bm_ps[:], gbb[:, cc * 128:(cc + 1) * 128],
                        aps, start=True, stop=False)
                    pend.append((s, cc, pp, bm_ps, tmps[cc]))
                    if len(pend) > 1:
                        finish_piece(*pend.pop(0))

            def stats_thunks(s):
                th = []
                for cc in range(NC4):
                    for k in range(8):
                        th.append(lambda s=s, cc=cc, k=k: stats_slice(s, cc, k))
                    th.append(lambda s=s, cc=cc: stats_aggr(s, cc))
                return th

            # ---- emission schedule: s1 prologue interleaved into s0 pieces --
            prologue_scores(0)
            for t in stats_thunks(0):
                t()
            prologue_fold(0)
            side = stats_thunks(1)
            for pp in range(NP):
                if pp >= 1:
                    do_maps(0, pp - 1, side if pp >= 3 else None)
                do_attn(0, pp)
                if pp == 3:
                    prologue_scores(1)
            do_maps(0, NP - 1, side)
            while side:
                side.pop(0)()
            while st[0]["pend"]:
                finish_piece(*st[0]["pend"].pop(0))
            st[1]["attn_sb"] = attnpool.tile([L, HW], bf16, tag="attn_sb",
                                             name="attn_sb1")
            for pp in range(4):
                do_attn(1, pp)
            prologue_fold(1)
            for pp in range(4, NP):
                do_maps(1, pp - 4)
                do_attn(1, pp)
            for pp in range(4, NP):
                do_maps(1, pp)
            while st[1]["pend"]:
                finish_piece(*st[1]["pend"].pop(0))

    nc.compile()
    return nc


_NC_CACHE = None


def _get_nc():
    global _NC_CACHE
    if _NC_CACHE is None:
        _NC_CACHE = _build_program()
    return _NC_CACHE


def make_in_maps(inputs):
    import ml_dtypes
    f8 = np.float64
    bfd = ml_dtypes.bfloat16
    h = np.ascontiguousarray(inputs["h"], dtype=np.float32)
    ws = np.asarray(inputs["w_source"], dtype=np.float32)
    wt = np.asarray(inputs["w_target"], dtype=np.float32)
    conv_w = np.asarray(inputs["conv_w"], dtype=np.float32)
    conv_b = np.asarray(inputs["conv_b"], dtype=np.float32)
    fc_k_w = np.asarray(inputs["fc_k_w"], dtype=np.float32)
    fc_k_b = np.asarray(inputs["fc_k_b"], dtype=np.float32)
    fc_w = np.asarray(inputs["fc_w"], dtype=np.float32)
    fc_b = np.asarray(inputs["fc_b"], dtype=np.float32)
    in_w = np.asarray(inputs["in_w"], dtype=np.float32)
    in_b = np.asarray(inputs["in_b"], dtype=np.float32)

    ws_t = ws.transpose(0, 2, 1)                            # [B, D, L]
    ws_t4 = np.ascontiguousarray(
        ws_t.reshape(B, 4, 128, L).transpose(0, 2, 1, 3).reshape(B, 128, 4 * L))
    ws_t4_bf = np.ascontiguousarray(ws_t4.astype(bfd))
    wt_t4 = np.ascontiguousarray(
        wt.transpose(0, 2, 1).reshape(B, 4, 128, L)
        .transpose(0, 2, 1, 3).reshape(B, 128, 4 * L))
    wt_bf = np.ascontiguousarray(wt.astype(bfd))

    # host folds: Mf = fc_k_w.T @ conv_w ; c0 = conv_w.T @ fc_k_b ;
    # kb[b,l] = ws[b] @ (fc_k_w.T @ conv_b) + fc_k_b . conv_b
    cw2 = conv_w[:, :, 0, 0].astype(f8)                     # [Q, C]
    mf = (fc_k_w.astype(f8).T @ cw2).astype(np.float32)     # [D, C]
    c0 = (cw2.T @ fc_k_b.astype(f8)).astype(np.float32)     # [C]
    vb = fc_k_w.astype(f8).T @ conv_b.astype(f8)            # [D]
    kb = (ws.astype(f8) @ vb
          + fc_k_b.astype(f8) @ conv_b.astype(f8)).astype(np.float32)  # [B,L]

    # ---- packed weight block [128, 16128] bytes ----
    wblk = np.zeros((128, 16128), dtype=np.uint8)

    def put(col, arr, rows=128):
        b = np.ascontiguousarray(arr).view(np.uint8)
        b = b.reshape(rows, -1)
        wblk[:rows, col:col + b.shape[1]] = b
        return col + b.shape[1]

    mfb = mf.astype(bfd)
    for j in range(4):
        put(j * 1024, mfb[j * 128:(j + 1) * 128, :])
    fwb = fc_w.T.astype(bfd)
    for j in range(4):
        put(4096 + j * 2048, fwb[j * 128:(j + 1) * 128, :])
    put(14336, np.ones((L, L), dtype=np.float32), rows=L)
    put(14592, -np.eye(L, dtype=np.float32), rows=L)
    put(14848, np.eye(L, dtype=bfd), rows=L)
    put(15232, np.eye(128, dtype=bfd))
    put(15488, np.eye(128, dtype=np.float32))
    tiny = np.zeros((128, 16), dtype=np.float32)
    tiny[:, 0:4] = c0.reshape(4, 128).T
    tiny[:, 4:8] = in_w.reshape(4, 128).T
    tiny[:, 8:12] = in_b.reshape(4, 128).T
    tiny[:, 12] = EPS

    shared = {
        "fc_b_row": np.ascontiguousarray(fc_b.reshape(1, 2 * C)),
        "ones1": np.ones((1, L), dtype=np.float32),
        "sel8": np.repeat(np.eye(8, dtype=np.float32), L, axis=1),
    }
    in_maps = []
    for i in range(N_CORES):
        lo = i * S
        wb = wblk.copy()
        t = tiny.copy()
        t[0:64, 14] = kb[lo]
        t[0:64, 15] = kb[lo + 1]
        wb[:, 16000:16064] = t.view(np.uint8).reshape(128, 64)
        spk = np.zeros((S, 128, 3584), dtype=np.uint8)
        for s in range(S):
            b = lo + s
            spk[s, :, 0:1024] = ws_t4[b].view(np.uint8).reshape(128, 1024)
            spk[s, :, 1024:2048] = wt_t4[b].view(np.uint8).reshape(128, 1024)
            spk[s, :, 2048:2560] = ws_t4_bf[b].view(np.uint8).reshape(128, 512)
            spk[s, 0:64, 2560:3584] = wt_bf[b].view(np.uint8).reshape(64, 1024)
        in_maps.append({
            "h": h[lo:lo + S],
            "wblk": wb,
            "spk": spk,
            **shared,
        })
    return in_maps


def kernel(**inputs):
    in_maps = make_in_maps(inputs)
    nc = _get_nc()
    res = run_bass_kernel_spmd(nc, in_maps, core_ids=list(range(N_CORES)))
    out = np.concatenate(
        [np.asarray(res.results[i]["out"]) for i in range(N_CORES)], axis=0)
    return out.astype(np.float32).reshape(B, C, H, W)


if __name__ == "__main__":
    rng = np.random.default_rng(0)
    ins = {
        "h": rng.standard_normal((B, C, H, W), dtype=np.float32),
        "w_source": rng.standard_normal((B, L, D), dtype=np.float32),
        "w_target": rng.standard_normal((B, L, D), dtype=np.float32),
        "conv_w": (rng.standard_normal((Q, C, 1, 1), dtype=np.float32)
                   / np.sqrt(C)),
        "conv_b": np.zeros(Q, np.float32),
        "fc_k_w": (rng.standard_normal((Q, D), dtype=np.float32)
                   / np.sqrt(D)),
        "fc_k_b": np.zeros(Q, np.float32),
        "fc_w": (rng.standard_normal((2 * C, D), dtype=np.float32)
                 / np.sqrt(D)),
        "fc_b": np.zeros(2 * C, np.float32),
        "in_w": np.ones(C, np.float32),
        "in_b": np.zeros(C, np.float32),
    }
    out = kernel(**ins)
    print("out", out.shape, out.dtype, float(np.abs(out).max()))


# revision 51
# speedup vs baseline: 1.0146x; 1.0146x over previous
"""Trainium2 Bass kernel for nn_MACAM (cross-attn modulation + instance norm).

Strategy: pure data parallel - batch B=16 sharded 2 samples per core over 8
NeuronCores.  Per sample the conv + fc_k are folded on the host into a single
matrix Mf = fc_k_w.T @ conv_w so the device computes
    kcT = Mf.T @ ws.T + c0,   attn = kcT.T @ h + kb        (kb host-folded)
The argmax/gather runs on-chip as a one-hot matmul.  The beta/gamma map
matmuls run with K=64 (attn lives on partitions 0-63), bf16 weights and
moving operand.  InstanceNorm scale `s` folds at piece level via
scalar_tensor_tensor  tmp = (gamma_map * s) * h  on DVE, and the final add
runs as a PE identity-accumulate into the beta-map PSUM (beta2 =
beta + t2*gamma folded on-chip).  Output is written bf16 and upcast on the
host.  attn pieces are interleaved with modulation pieces so the PE never
idles long enough for HAM to re-throttle the clock.
"""

import os
import sys

os.environ.setdefault("MYCRO_LOCAL_CACHE", "1")
sys.path.insert(0, "/opt/trn_rl_repo")

import numpy as np

import concourse.bacc as bacc
import concourse.bass as bass
import concourse.mybir as mybir
import concourse.tile as tile
from concourse.bass_utils import run_bass_kernel_spmd

N_CORES = 8
B, C, H, W = 16, 512, 64, 64
HW = H * W
L, D, Q = 64, 512, 512
S = B // N_CORES          # samples per core
EPS = 1e-5
NP = 8                    # HW pieces of 512
NC4 = 4                   # channel chunks of 128

f32 = mybir.dt.float32
f32r = mybir.dt.float32r
bf16 = mybir.dt.bfloat16
AF = mybir.ActivationFunctionType
ALU = mybir.AluOpType
AX = mybir.AxisListType


def _build_program():
    nc = bacc.Bacc("TRN2", target_bir_lowering=False, debug=False,
                   num_devices=N_CORES)
    dt_ = nc.dram_tensor
    h_d = dt_("h", [S, C, H, W], f32, kind="ExternalInput").ap()
    u8 = mybir.dt.uint8
    WB = 16128
    SB = 3584
    wblk_d = dt_("wblk", [128, WB], u8, kind="ExternalInput").ap()
    spk_d = dt_("spk", [S, 128, SB], u8, kind="ExternalInput").ap()
    fb_d = dt_("fc_b_row", [1, 2 * C], f32, kind="ExternalInput").ap()
    ones1_d = dt_("ones1", [1, L], f32, kind="ExternalInput").ap()
    sel8_d = dt_("sel8", [8, 8 * L], f32, kind="ExternalInput").ap()
    out_d = dt_("out", [S, C, HW], bf16, kind="ExternalOutput").ap()

    h_v = h_d.rearrange("s (n p) a b -> s n p (a b)", p=128)     # [S,4,128,4096]
    out_v = out_d.rearrange("s (n p) q -> s n p q", p=128)

    with tile.TileContext(nc) as tc:
        with (
            tc.tile_pool(name="wpool", bufs=1) as wpool,
            tc.tile_pool(name="hpool", bufs=8) as hpool,
            tc.tile_pool(name="attnpool", bufs=2) as attnpool,
            tc.tile_pool(name="spool", bufs=2) as spool,
            tc.tile_pool(name="piece", bufs=5) as piece,
            tc.tile_pool(name="ps_bg", bufs=1, space="PSUM") as ps_bg,
            tc.tile_pool(name="ps_ring", bufs=7, space="PSUM") as ps_ring,
        ):
            # ---- packed input DMAs: one big-packet block for the
            # persistent weights, one per-sample block for the smalls, so
            # they get a fair DMA round-robin share against the h stream --
            wblk = wpool.tile([128, WB], mybir.dt.uint8, tag="wblk")
            nc.sync.dma_start(wblk[:], wblk_d)
            mf_t = [wblk[:, j * 1024:(j + 1) * 1024].bitcast(bf16)
                    for j in range(4)]
            fw_t = [wblk[:, 4096 + j * 2048:4096 + (j + 1) * 2048].bitcast(bf16)
                    for j in range(4)]

            ones64 = wblk[0:L, 14336:14592].bitcast(f32)
            nid64 = wblk[0:L, 14592:14848].bitcast(f32)
            id64b = wblk[0:L, 14848:14976].bitcast(bf16)
            id128b = wblk[:, 15232:15488].bitcast(bf16)
            id128 = wblk[:, 15488:16000].bitcast(f32)
            tinyb = wblk[:, 16000:16064].bitcast(f32)
            c0_col = tinyb[:, 0:4]
            inw_col = tinyb[:, 4:8]
            inb_col = tinyb[:, 8:12]
            epz = tinyb[:, 12:14]
            kbs = [tinyb[0:L, 14:15], tinyb[0:L, 15:16]]
            wsT4s, wsT4bs, wtT4s, wts, hts = [], [], [], [], []
            for s in range(S):
                spk = spool.tile([128, SB], mybir.dt.uint8, tag="spk")
                nc.sync.dma_start(spk[:], spk_d[s])
                wsT4s.append(spk[:, 0:1024].bitcast(f32))
                wtT4s.append(spk[:, 1024:2048].bitcast(f32))
                wsT4bs.append(spk[:, 2048:2560].bitcast(bf16))
                wts.append(spk[0:L, 2560:3584].bitcast(bf16))
            fcb_row = wpool.tile([1, 2 * C], f32r, tag="fcb")
            nc.sync.dma_start(fcb_row[:], fb_d.bitcast(f32r))
            ones1r = wpool.tile([1, L], f32r, tag="ones1r")
            nc.sync.dma_start(ones1r[:], ones1_d.bitcast(f32r))
            sel8 = wpool.tile([8, 8 * L], f32r, tag="sel8")
            nc.sync.dma_start(sel8[:], sel8_d.bitcast(f32r))
            for s in range(S):
                h_t = []
                for cc in range(NC4):
                    t = hpool.tile([128, HW], bf16, tag="h", name=f"h{s}{cc}")
                    nc.gpsimd.dma_start(t[:], h_v[s, cc])
                    h_t.append(t)
                hts.append(h_t)

            st = [dict() for _ in range(S)]

            def prologue_scores(s):
                wsT4, wtT4, wt_sb = wsT4s[s], wtT4s[s], wts[s]
                # one PSUM bank holds the whole small path as scratch:
                #   [0:64, 0:64]    scores       [0:1, 64:128] colsum
                #   [0:64,128:192]  left         [0:64,192:224] PT (bf16)
                #   [0:8, 224:352]  stT          [0:128,384:448] kcT (4x)
                #   [0:128,448:512] waT (4x)
                # then gamma -> [0:64, 0:512], copied out, then beta.
                scr = ps_bg.tile([128, 512], f32, tag="bg")
                st[s]["scr"] = scr

                scores_ps = scr[0:L, 0:L]
                for j in range(4):
                    nc.tensor.matmul(
                        scores_ps,
                        wsT4[:, j * L:(j + 1) * L], wtT4[:, j * L:(j + 1) * L],
                        start=(j == 0), stop=(j == 3))
                scores_sb = spool.tile([L, L], f32, tag="scores_sb")
                nc.scalar.copy(scores_sb[:], scores_ps)
                colsum_ps = scr[0:1, 64:64 + L]
                nc.tensor.matmul(colsum_ps, ones64[:, 0:1], scores_sb[:],
                                 start=True, stop=True)
                colsum_row = spool.tile([1, L], f32, tag="colsum")
                nc.scalar.copy(colsum_row[:], colsum_ps)
                left_ps = scr[0:L, 128:128 + L]
                nc.tensor.matmul(left_ps, ones64[0:1, :], colsum_row[:],
                                 start=True, stop=False)
                nc.tensor.matmul(left_ps, nid64[:], scores_sb[:],
                                 start=False, stop=True)
                rowmax = spool.tile([L, 1], f32, tag="rowmax")
                nc.vector.tensor_reduce(rowmax[:], left_ps, AX.X, ALU.max)
                P_sb = spool.tile([L, L], bf16, tag="P_sb")
                nc.vector.tensor_scalar(P_sb[:], left_ps, rowmax[:], None,
                                        ALU.is_equal)
                PT_ps = scr[0:L, 192:224].bitcast(bf16)
                nc.tensor.transpose(PT_ps, P_sb[:], id64b[:])
                PT_sb = spool.tile([L, L], bf16, tag="PT_sb")
                nc.scalar.copy(PT_sb[:], PT_ps)

                # kcT = Mf.T @ ws.T + c0
                kcT_sb = spool.tile([128, L * NC4], bf16, tag="kcT_sb")
                st[s]["kcT"] = kcT_sb
                wsT4b = wsT4bs[s]
                for cc in range(NC4):
                    kcT_ps = scr[0:128, 384:384 + L]
                    for j in range(4):
                        nc.tensor.matmul(
                            kcT_ps, mf_t[j][:, cc * 128:(cc + 1) * 128],
                            wsT4b[:, j * L:(j + 1) * L],
                            start=(j == 0), stop=(j == 3))
                    nc.scalar.activation(
                        kcT_sb[:, cc * L:(cc + 1) * L],
                        kcT_ps, AF.Identity, bias=c0_col[:, cc:cc + 1])

                # w_allocT
                waT_sb = spool.tile([128, 4 * L], bf16, tag="waT_sb")
                st[s]["waT"] = waT_sb
                for j in range(4):
                    waT_ps = scr[0:128, 448:448 + L]
                    nc.tensor.matmul(waT_ps, wt_sb[:, j * 128:(j + 1) * 128],
                                     PT_sb[:], start=True, stop=True)
                    nc.scalar.copy(waT_sb[:, j * L:(j + 1) * L], waT_ps)

            def stats_slice(s, cc, k):
                if "st_col" not in st[s]:
                    st[s]["st_col"] = spool.tile([128, 8], f32, tag="st_col", name="st_col")
                    st[s]["st6"] = {}
                if cc not in st[s]["st6"]:
                    st[s]["st6"][cc] = spool.tile([128, 48], f32, tag="st6", name="st6")
                st6 = st[s]["st6"][cc]
                nc.vector.bn_stats(
                    st6[:, k * 6:(k + 1) * 6],
                    hts[s][cc][:, k * 512:(k + 1) * 512])

            def stats_aggr(s, cc):
                st_col, st6 = st[s]["st_col"], st[s]["st6"][cc]
                mv = spool.tile([128, 2], f32, tag="mv")
                nc.vector.bn_aggr(mv[:], st6[:])
                sd = spool.tile([128, 1], f32, tag="sd")
                nc.scalar.activation(sd[:], mv[:, 1:2], AF.Sqrt, bias=epz[:, 0:1])
                rs = spool.tile([128, 1], f32, tag="rs")
                nc.vector.reciprocal(rs[:], sd[:])
                nc.vector.tensor_tensor(
                    st_col[:, cc:cc + 1], rs[:], inw_col[:, cc:cc + 1],
                    ALU.mult)
                ms = spool.tile([128, 1], f32, tag="ms")
                nc.vector.tensor_tensor(ms[:], mv[:, 0:1],
                                        st_col[:, cc:cc + 1], ALU.mult)
                nc.vector.tensor_tensor(st_col[:, 4 + cc:5 + cc],
                                        inb_col[:, cc:cc + 1], ms[:],
                                        ALU.subtract)

            def prologue_fold(s):
                scr, st_col, waT_sb = st[s]["scr"], st[s]["st_col"], st[s]["waT"]
                # t2 broadcast to [64,512]
                stT_ps = scr[0:8, 224:352]
                nc.tensor.transpose(stT_ps, st_col[:], id128[:])
                st8r = spool.tile([8, 128], f32r, tag="st8r")
                nc.scalar.copy(st8r[:], stT_ps)
                t2m_t = ps_ring.tile([128, 512], f32, tag="ring")
                t2m_ps = t2m_t[0:L, :]
                for j in range(4):
                    nc.tensor.matmul(t2m_ps[:, j * 128:(j + 1) * 128],
                                     sel8[:, (4 + j) * L:(5 + j) * L], st8r[:],
                                     start=True, stop=True)
                t2m_sb = spool.tile([L, C], bf16, tag="t2m_sb")
                nc.scalar.copy(t2m_sb[:], t2m_ps)
                smap_t = ps_ring.tile([128, 512], f32, tag="ring")
                smap_ps = smap_t[0:L, :]
                for j in range(4):
                    nc.tensor.matmul(smap_ps[:, j * 128:(j + 1) * 128],
                                     sel8[:, j * L:(j + 1) * L], st8r[:],
                                     start=True, stop=True)
                smap_sb = spool.tile([L, C], bf16, tag="smap_sb")
                nc.scalar.copy(smap_sb[:], smap_ps)

                # gamma then beta, sequentially through scr[0:64, :]
                for j in range(4):
                    nc.tensor.matmul(
                        scr[0:L, :], waT_sb[:, j * L:(j + 1) * L],
                        fw_t[j][:, C:2 * C], start=(j == 0), stop=False)
                nc.tensor.matmul(scr[0:L, :], ones1r[:], fcb_row[:, C:2 * C],
                                 start=False, stop=True)
                # gamma2 = s * gamma ;  tg = t2 * gamma  (both from PSUM)
                gbg = spool.tile([L, C], bf16, tag="gbg")
                nc.vector.tensor_tensor(gbg[:], scr[0:L, :], smap_sb[:],
                                        ALU.mult)
                st[s]["gbg"] = gbg
                nc.vector.tensor_tensor(t2m_sb[:], scr[0:L, :], t2m_sb[:],
                                        ALU.mult)
                for j in range(4):
                    nc.tensor.matmul(
                        scr[0:L, :], waT_sb[:, j * L:(j + 1) * L],
                        fw_t[j][:, 0:C], start=(j == 0), stop=False)
                nc.tensor.matmul(scr[0:L, :], ones1r[:], fcb_row[:, 0:C],
                                 start=False, stop=True)
                # beta2 = beta + t2*gamma
                gbb = spool.tile([L, C], bf16, tag="gbb")   # beta2
                nc.vector.tensor_tensor(gbb[:], scr[0:L, :], t2m_sb[:], ALU.add)
                st[s]["gbb"] = gbb
                if "attn_sb" not in st[s]:
                    st[s]["attn_sb"] = attnpool.tile(
                        [L, HW], bf16, tag="attn_sb", name="attn_sb")
                st[s]["pend"] = []

            def do_attn(s, pp):
                kcT_sb, h_t, attn_sb = st[s]["kcT"], hts[s], st[s]["attn_sb"]
                attn_t = ps_ring.tile([128, 512], f32, tag="ring")
                attn_ps = attn_t[0:L, :]
                for cc in range(NC4):
                    nc.tensor.matmul(
                        attn_ps, kcT_sb[:, cc * L:(cc + 1) * L],
                        h_t[cc][:, pp * 512:(pp + 1) * 512],
                        start=(cc == 0), stop=(cc == 3))
                nc.scalar.activation(attn_sb[:, pp * 512:(pp + 1) * 512],
                                     attn_ps, AF.Identity, bias=kbs[s][:])

            def finish_piece(s, cc, pp, bm_ps, tmp):
                nc.tensor.matmul(bm_ps[:], id128b[:], tmp[:],
                                 start=False, stop=True)
                outp = piece.tile([128, 512], bf16, tag="outp")
                nc.scalar.copy(outp[:], bm_ps[:])
                nc.sync.dma_start(
                    out_v[s, cc][:, pp * 512:(pp + 1) * 512], outp[:])

            def do_maps(s, pp, side=None):
                attn_sb, gbg, gbb = st[s]["attn_sb"], st[s]["gbg"], st[s]["gbb"]
                h_t, pend = hts[s], st[s]["pend"]
                aps = attn_sb[:, pp * 512:(pp + 1) * 512]
                # phase 1: all gamma-map matmuls + vector multiplies, so the
                # multiplies overlap the beta-map matmuls of phase 2
                gms, tmps = [], []
                for cc in range(NC4):
                    gm_ps = ps_ring.tile([128, 512], f32, tag="ring")
                    nc.tensor.matmul(
                        gm_ps[:], gbg[:, cc * 128:(cc + 1) * 128],
                        aps, start=True, stop=True)
                    tmp = piece.tile([128, 512], bf16, tag="tmp")
                    if cc == 3:
                        gmc = piece.tile([128, 512], bf16, tag="gmc")
                        nc.scalar.copy(gmc[:], gm_ps[:])
                        nc.gpsimd.tensor_tensor(
                            tmp[:], gmc[:],
                            h_t[cc][:, pp * 512:(pp + 1) * 512], ALU.mult)
                    else:
                        nc.vector.tensor_tensor(
                            tmp[:], gm_ps[:],
                            h_t[cc][:, pp * 512:(pp + 1) * 512], ALU.mult)
                    tmps.append(tmp)
                    if side:
                        side.pop(0)()
                        if side and cc % 2 == 1:
                            side.pop(0)()
                # phase 2: beta-map matmuls + accumulate + copy out
                for cc in range(NC4):
                    bm_ps = ps_ring.tile([128, 512], f32, tag="ring")
                    nc.tensor.matmul(
                        bm_ps[:], gbb[:, cc * 128:(cc + 1) * 128],
                        aps, start=True, stop=False)
                    pend.append((s, cc, pp, bm_ps, tmps[cc]))
                    if len(pend) > 1:
                        finish_piece(*pend.pop(0))

            def stats_thunks(s):
                th = []
                for cc in range(NC4):
                    for k in range(8):
                        th.append(lambda s=s, cc=cc, k=k: stats_slice(s, cc, k))
                    th.append(lambda s=s, cc=cc: stats_aggr(s, cc))
                return th

            # ---- emission schedule: s1 prologue interleaved into s0 pieces --
            prologue_scores(0)
            for t in stats_thunks(0):
                t()
            prologue_fold(0)
            side = stats_thunks(1)
            for pp in range(NP):
                if pp >= 1:
                    do_maps(0, pp - 1, side if pp >= 4 else None)
                do_attn(0, pp)
                if pp == 3:
                    prologue_scores(1)
            do_maps(0, NP - 1, side)
            while side:
                side.pop(0)()
            while st[0]["pend"]:
                finish_piece(*st[0]["pend"].pop(0))
            st[1]["attn_sb"] = attnpool.tile([L, HW], bf16, tag="attn_sb",
                                             name="attn_sb1")
            for pp in range(4):
                do_attn(1, pp)
            prologue_fold(1)
            for pp in range(4, NP):
                do_maps(1, pp - 4)
                do_attn(1, pp)
            for pp in range(4, NP):
                do_maps(1, pp)
            while st[1]["pend"]:
                finish_piece(*st[1]["pend"].pop(0))

    nc.compile()
    return nc


_NC_CACHE = None


def _get_nc():
    global _NC_CACHE
    if _NC_CACHE is None:
        _NC_CACHE = _build_program()
    return _NC_CACHE


def make_in_maps(inputs):
    import ml_dtypes
    f8 = np.float64
    bfd = ml_dtypes.bfloat16
    h = np.ascontiguousarray(inputs["h"], dtype=np.float32)
    ws = np.asarray(inputs["w_source"], dtype=np.float32)
    wt = np.asarray(inputs["w_target"], dtype=np.float32)
    conv_w = np.asarray(inputs["conv_w"], dtype=np.float32)
    conv_b = np.asarray(inputs["conv_b"], dtype=np.float32)
    fc_k_w = np.asarray(inputs["fc_k_w"], dtype=np.float32)
    fc_k_b = np.asarray(inputs["fc_k_b"], dtype=np.float32)
    fc_w = np.asarray(inputs["fc_w"], dtype=np.float32)
    fc_b = np.asarray(inputs["fc_b"], dtype=np.float32)
    in_w = np.asarray(inputs["in_w"], dtype=np.float32)
    in_b = np.asarray(inputs["in_b"], dtype=np.float32)

    ws_t = ws.transpose(0, 2, 1)                            # [B, D, L]
    ws_t4 = np.ascontiguousarray(
        ws_t.reshape(B, 4, 128, L).transpose(0, 2, 1, 3).reshape(B, 128, 4 * L))
    ws_t4_bf = np.ascontiguousarray(ws_t4.astype(bfd))
    wt_t4 = np.ascontiguousarray(
        wt.transpose(0, 2, 1).reshape(B, 4, 128, L)
        .transpose(0, 2, 1, 3).reshape(B, 128, 4 * L))
    wt_bf = np.ascontiguousarray(wt.astype(bfd))

    # host folds: Mf = fc_k_w.T @ conv_w ; c0 = conv_w.T @ fc_k_b ;
    # kb[b,l] = ws[b] @ (fc_k_w.T @ conv_b) + fc_k_b . conv_b
    cw2 = conv_w[:, :, 0, 0].astype(f8)                     # [Q, C]
    mf = (fc_k_w.astype(f8).T @ cw2).astype(np.float32)     # [D, C]
    c0 = (cw2.T @ fc_k_b.astype(f8)).astype(np.float32)     # [C]
    vb = fc_k_w.astype(f8).T @ conv_b.astype(f8)            # [D]
    kb = (ws.astype(f8) @ vb
          + fc_k_b.astype(f8) @ conv_b.astype(f8)).astype(np.float32)  # [B,L]

    # ---- packed weight block [128, 16128] bytes ----
    wblk = np.zeros((128, 16128), dtype=np.uint8)

    def put(col, arr, rows=128):
        b = np.ascontiguousarray(arr).view(np.uint8)
        b = b.reshape(rows, -1)
        wblk[:rows, col:col + b.shape[1]] = b
        return col + b.shape[1]

    mfb = mf.astype(bfd)
    for j in range(4):
        put(j * 1024, mfb[j * 128:(j + 1) * 128, :])
    fwb = fc_w.T.astype(bfd)
    for j in range(4):
        put(4096 + j * 2048, fwb[j * 128:(j + 1) * 128, :])
    put(14336, np.ones((L, L), dtype=np.float32), rows=L)
    put(14592, -np.eye(L, dtype=np.float32), rows=L)
    put(14848, np.eye(L, dtype=bfd), rows=L)
    put(15232, np.eye(128, dtype=bfd))
    put(15488, np.eye(128, dtype=np.float32))
    tiny = np.zeros((128, 16), dtype=np.float32)
    tiny[:, 0:4] = c0.reshape(4, 128).T
    tiny[:, 4:8] = in_w.reshape(4, 128).T
    tiny[:, 8:12] = in_b.reshape(4, 128).T
    tiny[:, 12] = EPS

    shared = {
        "fc_b_row": np.ascontiguousarray(fc_b.reshape(1, 2 * C)),
        "ones1": np.ones((1, L), dtype=np.float32),
        "sel8": np.repeat(np.eye(8, dtype=np.float32), L, axis=1),
    }
    in_maps = []
    for i in range(N_CORES):
        lo = i * S
        wb = wblk.copy()
        t = tiny.copy()
        t[0:64, 14] = kb[lo]
        t[0:64, 15] = kb[lo + 1]
        wb[:, 16000:16064] = t.view(np.uint8).reshape(128, 64)
        spk = np.zeros((S, 128, 3584), dtype=np.uint8)
        for s in range(S):
            b = lo + s
            spk[s, :, 0:1024] = ws_t4[b].view(np.uint8).reshape(128, 1024)
            spk[s, :, 1024:2048] = wt_t4[b].view(np.uint8).reshape(128, 1024)
            spk[s, :, 2048:2560] = ws_t4_bf[b].view(np.uint8).reshape(128, 512)
            spk[s, 0:64, 2560:3584] = wt_bf[b].view(np.uint8).reshape(64, 1024)
        in_maps.append({
            "h": h[lo:lo + S],
            "wblk": wb,
            "spk": spk,
            **shared,
        })
    return in_maps


def kernel(**inputs):
    in_maps = make_in_maps(inputs)
    nc = _get_nc()
    res = run_bass_kernel_spmd(nc, in_maps, core_ids=list(range(N_CORES)))
    out = np.concatenate(
        [np.asarray(res.results[i]["out"]) for i in range(N_CORES)], axis=0)
    return out.astype(np.float32).reshape(B, C, H, W)


if __name__ == "__main__":
    rng = np.random.default_rng(0)
    ins = {
        "h": rng.standard_normal((B, C, H, W), dtype=np.float32),
        "w_source": rng.standard_normal((B, L, D), dtype=np.float32),
        "w_target": rng.standard_normal((B, L, D), dtype=np.float32),
        "conv_w": (rng.standard_normal((Q, C, 1, 1), dtype=np.float32)
                   / np.sqrt(C)),
        "conv_b": np.zeros(Q, np.float32),
        "fc_k_w": (rng.standard_normal((Q, D), dtype=np.float32)
                   / np.sqrt(D)),
        "fc_k_b": np.zeros(Q, np.float32),
        "fc_w": (rng.standard_normal((2 * C, D), dtype=np.float32)
                 / np.sqrt(D)),
        "fc_b": np.zeros(2 * C, np.float32),
        "in_w": np.ones(C, np.float32),
        "in_b": np.zeros(C, np.float32),
    }
    out = kernel(**ins)
    print("out", out.shape, out.dtype, float(np.abs(out).max()))
